# revision 33
# baseline (speedup 1.0000x reference)
"""T5-style encoder layer (pre-LN, RMSNorm, relative-position bias) on 8 trn2
NeuronCores, data-parallel over the batch dimension (B=8 -> one batch element
per core). Each core runs the full layer for its [S, D] slice; weights and the
relative-bias diagonal blocks are replicated.

Self-contained: hardcodes all shapes; only depends on the runtime at
/opt/trn_rl_repo.
"""

import sys

if "/opt/trn_rl_repo" not in sys.path:
    sys.path.insert(0, "/opt/trn_rl_repo")

import numpy as np
import ml_dtypes

import concourse.bass as bass
import concourse.tile as tile
from concourse import bacc
from concourse import mybir
from concourse.bass_utils import run_bass_kernel_spmd
from concourse.masks import make_identity

# ---- problem constants -----------------------------------------------------
B, S, D = 8, 1024, 1024
H, HD = 16, 64
MLP = 4096
NUM_BUCKETS, MAX_DIST = 32, 128
EPS = 1e-6
NCORES = 8
P = 128
NS = S // P        # 8 token tiles
ND = D // P        # 8 feature tiles
NM = MLP // P      # 32 mlp tiles
NDIAG = 2 * NS - 1  # 15 distinct 128x128 tile-diagonals of the bias

F32 = mybir.dt.float32
F32R = mybir.dt.float32r
BF16 = mybir.dt.bfloat16
BF16NP = ml_dtypes.bfloat16


# ---- host-side relative position bias --------------------------------------
def _rel_pos_bucket_np(rel):
    # mirrors t5x _relative_position_bucket (bidirectional), numpy fp32
    n = -rel
    num_buckets = NUM_BUCKETS // 2          # 16
    ret = (n < 0).astype(np.int32) * num_buckets
    n = np.abs(n)
    max_exact = num_buckets // 2            # 8
    is_small = n < max_exact
    val_if_large = max_exact + (
        np.log(n.astype(np.float32) / max_exact + np.finfo(np.float32).eps)
        / np.log(MAX_DIST / max_exact)
        * (num_buckets - max_exact)
    ).astype(np.int32)
    val_if_large = np.minimum(val_if_large, num_buckets - 1)
    return ret + np.where(is_small, n, val_if_large)


def _bias_blocks(rel_emb):
    """[H, 128, NDIAG, 128] f32 blocks of the transposed bias.

    Block d' (=7-m, m = k_tile - q_tile) at [p, c] = bias^T[k, q] for
    k = k_tile*128 + p, q = q_tile*128 + c, i.e. table[1023 + m*128 + p - c].
    """
    rel = np.arange(-(S - 1), S, dtype=np.int32)          # k - q in [-1023, 1023]
    buckets = _rel_pos_bucket_np(rel)                     # [2047]
    table = rel_emb[buckets, :].astype(np.float32)        # [2047, H]
    pp = np.arange(P)[:, None, None]
    dd = np.arange(NDIAG)[None, :, None]
    cc = np.arange(P)[None, None, :]
    idx = 1023 + (NS - 1 - dd) * P + pp - cc              # [128, NDIAG, 128]
    blocks = np.exp(table[idx])                           # [128, NDIAG, 128, H]
    return np.ascontiguousarray(blocks.transpose(3, 0, 1, 2)).astype(BF16NP)


# ---- device kernel ---------------------------------------------------------
def build_nc():
    nc = bacc.Bacc(None, target_bir_lowering=False)

    x_d = nc.declare_dram_parameter("x", [S, D], F32, isOutput=False)
    wq_d = nc.declare_dram_parameter("wq", [D, H * HD], F32R, isOutput=False)
    wk_d = nc.declare_dram_parameter("wk", [D, H * HD], F32R, isOutput=False)
    wv_d = nc.declare_dram_parameter("wv", [D, H * HD], F32R, isOutput=False)
    wo_d = nc.declare_dram_parameter("wo", [H * HD, D], BF16, isOutput=False)
    wi_d = nc.declare_dram_parameter("wi", [D, MLP], BF16, isOutput=False)
    wm_d = nc.declare_dram_parameter("womlp", [MLP, D], BF16, isOutput=False)
    bias_d = nc.declare_dram_parameter("biasb", [H, P, NDIAG, P], BF16, isOutput=False)
    out_d = nc.declare_dram_parameter("out", [S, D], F32, isOutput=True)
    rden_scr = nc.dram_tensor("rden_scr", [H, S], F32)

    wo_t = wo_d.ap().rearrange("(hp p) d -> p hp d", p=P)
    wq_t = wq_d.ap().rearrange("(di p) m -> p di m", p=P)
    wk_t = wk_d.ap().rearrange("(di p) m -> p di m", p=P)
    wv_t = wv_d.ap().rearrange("(di p) m -> p di m", p=P)
    wi_t = wi_d.ap().rearrange("(di p) m -> p di m", p=P)
    wm_t = wm_d.ap().rearrange("(ci p) d -> p ci d", p=P)

    with tile.TileContext(nc) as tc:
        _body(nc, tc, x_d, wq_t, wk_t, wv_t, wo_t, wi_t, wm_t, bias_d, out_d, rden_scr)
    nc.finalize()
    return nc


def _rmsnorm(nc, pools, src_ap, dst_tile, eps_t):
    """dst = src * rsqrt(mean(src^2) + eps); src [128, D] f32, dst any dtype.

    dst is also used as scratch for the squared values before the final write.
    """
    var = pools["nrm"].tile([P, 1], F32, tag="var")
    nc.vector.tensor_mul(out=dst_tile, in0=src_ap, in1=src_ap)
    nc.vector.reduce_sum(out=var, in_=dst_tile[:, :], axis=mybir.AxisListType.X)
    sd = pools["nrm"].tile([P, 1], F32, tag="sd")
    nc.scalar.activation(out=sd, in_=var, func=mybir.ActivationFunctionType.Sqrt,
                         bias=eps_t[:, :], scale=1.0 / D)
    rstd = pools["nrm"].tile([P, 1], F32, tag="rstd")
    nc.vector.reciprocal(out=rstd, in_=sd)
    nc.scalar.activation(out=dst_tile, in_=src_ap,
                         func=mybir.ActivationFunctionType.Copy,
                         bias=0.0, scale=rstd[:, :])


def _transpose_into(nc, psum_pool, src_tile, dst, si, ident):
    """PE-transpose [128, D] f32/bf16 src into dst[:, di, si*128:...]."""
    for di in range(ND):
        ps = psum_pool.tile([P, P], F32, space="PSUM", tag="tp")
        nc.tensor.transpose(ps[:, :], src_tile[:, di * P:(di + 1) * P], ident[:, :])
        nc.scalar.copy(out=dst[:, di, si * P:(si + 1) * P], in_=ps[:, :])


def _body(nc, tc, x_d, wq_t, wk_t, wv_t, wo_t, wi_t, wm_t, bias_d, out_d, rden_scr):
    fp = {}  # pools

    def pool(name, bufs, space="SBUF"):
        p = tc.alloc_tile_pool(name=name, bufs=bufs, space=space)
        fp[name] = p
        return p

    AF = mybir.ActivationFunctionType
    ALU = mybir.AluOpType

    singles = pool("singles", 1)
    ident32 = singles.tile([P, P], F32)
    make_identity(nc, ident32)
    ident16 = singles.tile([P, P], BF16)
    make_identity(nc, ident16)
    eps_t = singles.tile([P, 1], F32)
    nc.vector.memset(eps_t, EPS)

    pool("sc", 2)      # [128, D] scratch
    pool("nrm", 8)     # [128, 1] norm scalars
    pool("xs", 2)      # x stream tiles

    # activations that live through the attention block
    qkv_act = tc.alloc_tile_pool(name="qkv_act", bufs=1)
    qT = qkv_act.tile([P, ND, S], F32R)     # q^T  [hhd, s]
    kT = qkv_act.tile([P, ND, S], F32R)     # k^T  [hhd, s]
    v_ext = qkv_act.tile([P, NS, H, HD + 1], BF16)  # [tok, stile, h, hd|1]

    nc.vector.memset(v_ext[:, :, :, HD:HD + 1], 1.0)

    # ---- stage 1: rmsnorm(x) -> hT (feature-major) -------------------------
    with tc.tile_pool(name="hT_pool", bufs=1) as hT_pool:
        hT = hT_pool.tile([P, ND, S], F32R)
        with tc.tile_pool(name="tp1", bufs=4, space="PSUM") as tp1:
            for si in range(NS):
                xt = fp["xs"].tile([P, D], F32, tag="x")
                nc.sync.dma_start(out=xt, in_=x_d.ap()[si * P:(si + 1) * P, :])
                ht = fp["sc"].tile([P, D], F32, tag="h")
                _rmsnorm(nc, fp, xt[:, :], ht, eps_t)
                _transpose_into(nc, tp1, ht, hT, si, ident32)

        # ---- stage 2: QKV projections (fp32r) -------------------------------
        with tc.tile_pool(name="wqkv", bufs=2) as wqkv, \
             tc.tile_pool(name="psqkv", bufs=2, space="PSUM") as psqkv, \
             tc.tile_pool(name="psv", bufs=2, space="PSUM") as psv:
            for (w_ap, dstT) in ((wq_t, qT), (wk_t, kT)):
                for half in range(2):
                    w_sb = wqkv.tile([P, ND, 512], F32R, tag="w")
                    nc.sync.dma_start(out=w_sb, in_=w_ap[:, :, half * 512:(half + 1) * 512])
                    for mj in range(4):
                        m0 = half * 4 + mj
                        ps = psqkv.tile([P, S], F32, space="PSUM", tag="qkv")
                        for di in range(ND):
                            for sh in range(2):
                                nc.tensor.matmul(
                                    ps[:, sh * 512:(sh + 1) * 512],
                                    w_sb[:, di, mj * P:(mj + 1) * P],
                                    hT[:, di, sh * 512:(sh + 1) * 512],
                                    start=(di == 0), stop=(di == ND - 1),
                                )
                        nc.vector.tensor_copy(out=dstT[:, m0, :], in_=ps[:, :])
            # v: token-major, written into v_ext with the ones column gap
            for half in range(2):
                w_sb = wqkv.tile([P, ND, 512], F32R, tag="w")
                nc.sync.dma_start(out=w_sb, in_=wv_t[:, :, half * 512:(half + 1) * 512])
                for ci in range(NS):
                    ps = psv.tile([P, 512], F32, space="PSUM", tag="vps")
                    for di in range(ND):
                        nc.tensor.matmul(
                            ps[:, :],
                            hT[:, di, ci * P:(ci + 1) * P],
                            w_sb[:, di, :],
                            start=(di == 0), stop=(di == ND - 1),
                        )
                    nc.scalar.copy(
                        out=v_ext[:, ci, half * 8:half * 8 + 8, 0:HD],
                        in_=ps[:, :].rearrange("p (h e) -> p h e", e=HD),
                    )

    # ---- stage 3: attention per head ---------------------------------------
    attnT_pool = tc.alloc_tile_pool(name="attnT_pool", bufs=1)
    # attn^T packed: head 2i on partitions 0-63, head 2i+1 on 64-127
    attnT = attnT_pool.tile([P, H // 2, S], BF16)
    with (
        tc.tile_pool(name="biasp", bufs=2) as biasp,
        tc.tile_pool(name="wexpp", bufs=6) as wexpp,
        tc.tile_pool(name="lgp", bufs=2, space="PSUM") as lgp,
        tc.tile_pool(name="aup", bufs=2, space="PSUM") as aup,
        tc.tile_pool(name="rp", bufs=2) as rp,
    ):
        for h in range(H):
            hb = HD * (h % 2)           # partition base of this head in qT/kT
            hm = h // 2
            bias_sb = biasp.tile([P, NDIAG, P], BF16, tag="bias")
            nc.sync.dma_start(out=bias_sb, in_=bias_d.ap()[h])
            au = aup.tile([HD + 1, S], F32, tag="au")
            for ki in range(NS):
                lg = lgp.tile([P, S], F32, tag="lg")
                for qh in range(2):
                    nc.tensor.matmul(
                        lg[:, qh * 512:(qh + 1) * 512],
                        kT[hb:hb + HD, hm, ki * P:(ki + 1) * P],
                        qT[hb:hb + HD, hm, qh * 512:(qh + 1) * 512],
                        start=True, stop=True,
                    )
                # w = exp(l) * exp(bias): exp on ACT straight from PSUM,
                # then an all-bf16 SBUF multiply on DVE (2x mode)
                ex = wexpp.tile([P, S], BF16, tag="ex")
                nc.scalar.activation(out=ex, in_=lg[:, :], func=AF.Exp)
                wexp = wexpp.tile([P, S], BF16, tag="wexp")
                nc.vector.tensor_mul(
                    out=wexp[:, :].rearrange("p (c w) -> p c w", w=P),
                    in0=ex[:, :].rearrange("p (c w) -> p c w", w=P),
                    in1=bias_sb[:, NS - 1 - ki:2 * NS - 1 - ki, :],
                )
                for qh in range(2):
                    nc.tensor.matmul(
                        au[:, qh * 512:(qh + 1) * 512],
                        v_ext[:, ki, h, :],
                        wexp[:, qh * 512:(qh + 1) * 512],
                        start=(ki == 0), stop=(ki == NS - 1),
                    )
            rden = rp.tile([HD + 1, S], F32, tag="rden")
            nc.vector.reciprocal(out=rden[HD:HD + 1, :], in_=au[HD:HD + 1, :])
            # broadcast 1/denom to all 64 hd partitions via a DRAM bounce
            nc.sync.dma_start(out=rden_scr.ap()[h:h + 1, :], in_=rden[HD:HD + 1, :])
            rbc = rp.tile([HD, S], F32, tag="rbc")
            nc.sync.dma_start(out=rbc[:, :],
                              in_=rden_scr.ap()[h:h + 1, :].broadcast_to((HD, S)))
            hb2 = HD * (h % 2)
            nc.vector.tensor_mul(
                out=attnT[hb2:hb2 + HD, h // 2, :], in0=au[0:HD, :], in1=rbc[:, :],
            )

    # ---- stage 4: attn @ wo + residual -------------------------------------
    out1_pool = tc.alloc_tile_pool(name="out1_pool", bufs=1, side="right")
    out1 = out1_pool.tile([P, NS, D], F32)    # x + attn_out, token-major
    with tc.tile_pool(name="wop", bufs=1) as wop, \
         tc.tile_pool(name="ops", bufs=2, space="PSUM") as ops:
        wo_sb = wop.tile([P, H // 2, D], BF16)
        nc.sync.dma_start(out=wo_sb, in_=wo_t[:, :, :])
        for si in range(NS):
            ps = ops.tile([P, D], F32, tag="wo")
            for hp in range(H // 2):
                for dh in range(2):
                    nc.tensor.matmul(
                        ps[:, dh * 512:(dh + 1) * 512],
                        attnT[:, hp, si * P:(si + 1) * P],
                        wo_sb[:, hp, dh * 512:(dh + 1) * 512],
                        start=(hp == 0), stop=(hp == H // 2 - 1),
                    )
            xt = fp["xs"].tile([P, D], F32, tag="x")
            nc.sync.dma_start(out=xt, in_=x_d.ap()[si * P:(si + 1) * P, :])
            nc.vector.tensor_add(out=out1[:, si, :], in0=ps[:, :], in1=xt[:, :])
    attnT_pool.release()
    qkv_act.release()

    # ---- stage 5: rmsnorm(out1) -> h2T (bf16, feature-major) ---------------
    with tc.tile_pool(name="h2T_pool", bufs=1) as h2T_pool, \
         tc.tile_pool(name="yT_pool", bufs=1) as yT_pool:
        h2T = h2T_pool.tile([P, ND, S], BF16)
        with tc.tile_pool(name="tp5", bufs=4, space="PSUM") as tp5:
            for si in range(NS):
                h2 = fp["sc"].tile([P, D], BF16, tag="h2")
                _rmsnorm(nc, fp, out1[:, si, :], h2, eps_t)
                for di in range(ND):
                    ps = tp5.tile([P, P], BF16, space="PSUM", tag="tp16")
                    nc.tensor.transpose(ps[:, :], h2[:, di * P:(di + 1) * P], ident16[:, :])
                    nc.scalar.copy(out=h2T[:, di, si * P:(si + 1) * P], in_=ps[:, :])

        # ---- stage 6: y^T = relu(wi^T @ h2^T) (bf16) ------------------------
        yT = yT_pool.tile([P, NM, S], BF16)
        with tc.tile_pool(name="wip", bufs=2) as wip, \
             tc.tile_pool(name="psy", bufs=2, space="PSUM") as psy:
            for eighth in range(8):
                wi_sb = wip.tile([P, ND, MLP // 8], BF16, tag="wi")
                nc.sync.dma_start(out=wi_sb, in_=wi_t[:, :, eighth * (MLP // 8):(eighth + 1) * (MLP // 8)])
                for mj in range(NM // 8):
                    m0 = eighth * (NM // 8) + mj
                    ps = psy.tile([P, S], F32, space="PSUM", tag="y")
                    for di in range(ND):
                        for sh in range(2):
                            nc.tensor.matmul(
                                ps[:, sh * 512:(sh + 1) * 512],
                                wi_sb[:, di, mj * P:(mj + 1) * P],
                                h2T[:, di, sh * 512:(sh + 1) * 512],
                                start=(di == 0), stop=(di == ND - 1),
                            )
                    nc.scalar.activation(out=yT[:, m0, :], in_=ps[:, :], func=AF.Relu)

        # ---- stage 7: out = out1 + y^T.T @ womlp ----------------------------
        # womlp is streamed per 128-row chunk; four output tiles accumulate
        # concurrently (8 PSUM banks), so womlp is read twice overall.
        with tc.tile_pool(name="wmp", bufs=3) as wmp, \
             tc.tile_pool(name="o2ps", bufs=4, space="PSUM") as o2ps:
            for sg in range(2):
                pss = [o2ps.tile([P, D], F32, tag="o2", name=f"o2_{sg}_{i}") for i in range(4)]
                for ci in range(NM):
                    wmc = wmp.tile([P, D], BF16, tag="wm")
                    nc.sync.dma_start(out=wmc, in_=wm_t[:, ci, :])
                    for i4 in range(4):
                        si = sg * 4 + i4
                        for dh in range(2):
                            nc.tensor.matmul(
                                pss[i4][:, dh * 512:(dh + 1) * 512],
                                yT[:, ci, si * P:(si + 1) * P],
                                wmc[:, dh * 512:(dh + 1) * 512],
                                start=(ci == 0), stop=(ci == NM - 1),
                            )
                for i4 in range(4):
                    si = sg * 4 + i4
                    oo = fp["sc"].tile([P, D], F32, tag="oo")
                    nc.vector.tensor_add(out=oo, in0=pss[i4][:, :], in1=out1[:, si, :])
                    nc.sync.dma_start(out=out_d.ap()[si * P:(si + 1) * P, :], in_=oo)

    out1_pool.release()
    for name in ("xs", "nrm", "sc", "singles"):
        fp[name].release()


# ---- host wrapper ----------------------------------------------------------
_NC_CACHE = {}


def _get_nc():
    if "nc" not in _NC_CACHE:
        _NC_CACHE["nc"] = build_nc()
    return _NC_CACHE["nc"]


def _get_exec():
    """Compile once: a sharded PJRT executable over the 8 NeuronCores."""
    if "exec" in _NC_CACHE:
        return _NC_CACHE["exec"]
    import jax
    from jax.sharding import Mesh, PartitionSpec, NamedSharding
    from jax.experimental.shard_map import shard_map
    from concourse.bass2jax import (
        _bass_exec_p, install_neuronx_cc_hook, partition_id_tensor,
    )

    nc = _get_nc()
    install_neuronx_cc_hook()
    pname = nc.partition_id_tensor.name if nc.partition_id_tensor else None
    in_names, out_names, out_avals, zero_outs = [], [], [], []
    for alloc in nc.m.functions[0].allocations:
        if not isinstance(alloc, mybir.MemoryLocationSet):
            continue
        name = alloc.memorylocations[0].name
        if alloc.kind == "ExternalInput":
            if name != pname:
                in_names.append(name)
        elif alloc.kind == "ExternalOutput":
            out_names.append(name)
            shape = tuple(alloc.tensor_shape)
            dtype = mybir.dt.np(alloc.dtype)
            out_avals.append(jax.core.ShapedArray(shape, dtype))
            zero_outs.append(np.zeros(shape, dtype))
    n_params = len(in_names)
    all_in_names = in_names + out_names + ([pname] if pname else [])

    def _body(*args):
        operands = list(args)
        if pname is not None:
            operands.append(partition_id_tensor())
        outs = _bass_exec_p.bind(
            *operands,
            out_avals=tuple(out_avals),
            in_names=tuple(all_in_names),
            out_names=tuple(out_names),
            lowering_input_output_aliases=(),
            sim_require_finite=True,
            sim_require_nnan=True,
            nc=nc,
        )
        return tuple(outs)

    n_outs = len(out_avals)
    devices = jax.devices()[:NCORES]
    mesh = Mesh(np.asarray(devices), ("core",))
    sharded = jax.jit(
        shard_map(_body, mesh=mesh,
                  in_specs=(PartitionSpec("core"),) * (n_params + n_outs),
                  out_specs=(PartitionSpec("core"),) * n_outs,
                  check_rep=False),
        donate_argnums=tuple(range(n_params, n_params + n_outs)),
        keep_unused=True,
    )
    sh = NamedSharding(mesh, PartitionSpec("core"))
    _NC_CACHE["exec"] = (sharded, in_names, out_names, zero_outs, sh)
    return _NC_CACHE["exec"]


def _prep_inputs(x, ln1_scale, wq, wk, wv, wo_attn, ln2_scale, wi, wo_mlp, rel_emb):
    x = np.asarray(x, np.float32)
    ln1 = np.asarray(ln1_scale, np.float32)[:, None]
    ln2 = np.asarray(ln2_scale, np.float32)[:, None]
    wq_h = (np.asarray(wq, np.float32) * ln1).astype(np.float32)
    wk_h = (np.asarray(wk, np.float32) * ln1).astype(np.float32)
    wv_h = (np.asarray(wv, np.float32) * ln1).astype(np.float32)
    wo_h = np.asarray(wo_attn, np.float32).astype(BF16NP)
    wi_h = (np.asarray(wi, np.float32) * ln2).astype(BF16NP)
    wm_h = np.asarray(wo_mlp, np.float32).astype(BF16NP)
    biasb = _bias_blocks(np.asarray(rel_emb, np.float32))
    shared = {
        "wq": wq_h, "wk": wk_h, "wv": wv_h, "wo": wo_h,
        "wi": wi_h, "womlp": wm_h, "biasb": biasb,
    }
    in_maps = [dict(shared, x=np.ascontiguousarray(x[b])) for b in range(NCORES)]
    return in_maps


def kernel(x, ln1_scale, wq, wk, wv, wo_attn, ln2_scale, wi, wo_mlp, rel_emb):
    import jax
    in_maps = _prep_inputs(x, ln1_scale, wq, wk, wv, wo_attn, ln2_scale,
                           wi, wo_mlp, rel_emb)
    sharded, in_names, out_names, zero_outs, sh = _get_exec()
    concat_in = [
        jax.device_put(
            np.concatenate([in_maps[c][n] for c in range(NCORES)], axis=0), sh)
        for n in in_names
    ]
    czero = [
        jax.device_put(np.zeros((NCORES * z.shape[0], *z.shape[1:]), z.dtype), sh)
        for z in zero_outs
    ]
    outs = sharded(*concat_in, *czero)
    oidx = out_names.index("out")
    full = np.asarray(outs[oidx]).reshape(NCORES, S, D)
    return full.astype(np.float32)
